# revision 1
# baseline (speedup 1.0000x reference)
"""CLIP loss kernel for trn2, 8 NeuronCores, data-parallel over the batch dim.

Strategy (per core c of 8, SPMD):
  inputs: img slice [1024, 512] f32, spec slice [1024, 512] f32 (rows
  1024c..1024c+1023 of each modality).
  1. sumsq of both slices on ACT (Square + accum_out); 16/|row| =
     Sqrt(256 * reciprocal(ss)) (DVE reciprocal + ACT Sqrt). ACT uses
     exactly two table sets (sqrt preamble / exp main loop), both
     loaded off the critical path via warm-up activations.
  2. both modalities normalized on-device to bf16 (x16 fp8 prescale
     folded into the Sqrt scale), transposed via PE, cast to fp8 in the
     PSUM->SBUF staging copies. spec^T AllGathered in two chunks (chunk
     0 gates only the first half of the main loop; the first mesh
     cannot begin before the CC core's ~50-65us init anyway). The
     collective payload keeps rows = SBUF partitions, and specT uses an
     arrival-major layout [q, src core, k, off], so every unpack DMA
     moves 2KB-contiguous runs on both sides (the column permutation is
     harmless: the host only ever sums over all columns).
  3. logits block: out[m=img rows, n=spec cols] = imgT.T @ specT, fp8
     DoubleRow (K=256 per pass), PSUM f32, tiles [128, 2048].
  4. ACT Exp with constant scale = logit_scale/256; accum_out gives
     row-sums of exp for free. m=0 exp tile writes straight into
     racc[128, 8192] (bf16 column partials); m>0 tiles accumulated via
     DVE add. Final 128-partition column reduce happens on host.
  5. diag: raw img.spec dot per row (DVE, in the collective's shadow),
     combined with norms on host.
Host: gathers per-core row-sums / column partials / diag pieces, takes
logs and means (O(N) numpy) -> scalar loss.
"""

import os
from contextlib import ExitStack

import numpy as np

import concourse.bass as bass
import concourse.mybir as mybir
from concourse import bacc, tile
from concourse.bass_utils import run_bass_kernel_spmd
from concourse.masks import make_identity

N, D, C = 8192, 512, 8
NL = N // C  # 1024 local rows per core
P = 128
T = NL // P  # 8 natural [128, 512] tiles per modality slice
KC = D // P  # 4 contraction chunks
G = 4        # column groups per core block
GW = N // G  # 2048 columns per group

f32 = mybir.dt.float32
bf16 = mybir.dt.bfloat16
fp8 = mybir.dt.float8e4
FA = mybir.ActivationFunctionType
ALU = mybir.AluOpType

# fp8 operands are pre-scaled by 16 to stay out of the subnormal range;
# both sides carry the factor, compensated by scale/256 in the exp.
FP8_PRESCALE = 16.0

_cache: dict = {}

# dev bisection knobs (shipping defaults; NOTE tensor_tensor_reduce and
# fp8 PE transpose both fail on hardware — do not reintroduce them)
_USE_FP8T = os.environ.get("KERNEL_FP8T", "0") == "1"
_USE_DUMMY_CC = os.environ.get("KERNEL_DUMMY_CC", "0") == "1"


def _build(scale: float, use_cc: bool = True):
    nc = bacc.Bacc("TRN2", target_bir_lowering=False, debug=False, num_devices=C)
    img = nc.dram_tensor("img", [NL, D], bf16, kind="ExternalInput")
    spec = nc.dram_tensor("spec", [NL, D], bf16, kind="ExternalInput")
    rowsum_o = nc.dram_tensor("rowsum", [P, T], f32, kind="ExternalOutput")
    racc_o = nc.dram_tensor("racc_o", [P, N], bf16, kind="ExternalOutput")
    dotd_o = nc.dram_tensor("dotd", [P, T], f32, kind="ExternalOutput")
    rni_o = nc.dram_tensor("rni", [P, T], f32, kind="ExternalOutput")
    rns_o = nc.dram_tensor("rns", [P, T], f32, kind="ExternalOutput")

    tdt = fp8 if _USE_FP8T else bf16

    with tile.TileContext(nc) as tc, ExitStack() as ctx:
        const = ctx.enter_context(tc.tile_pool(name="const", bufs=1))
        natp = ctx.enter_context(tc.tile_pool(name="nat", bufs=T))
        scp = ctx.enter_context(tc.tile_pool(name="scr", bufs=2))
        spn = ctx.enter_context(tc.tile_pool(name="specn", bufs=4))
        pers = ctx.enter_context(tc.tile_pool(name="pers", bufs=1))
        ps = ctx.enter_context(tc.tile_pool(name="ps", bufs=2, space="PSUM"))
        ep = ctx.enter_context(tc.tile_pool(name="e", bufs=4))
        dramp = ctx.enter_context(tc.tile_pool(name="dram", bufs=1, space="DRAM"))

        if use_cc and _USE_DUMMY_CC:
            # First instruction of the program: trigger a tiny collective so
            # the CC core's ~30-40us init pipeline starts immediately. The
            # first real mesh cannot begin before that init completes.
            dmy_i = dramp.tile([1, 128], fp8, name="dmy_i")
            dmy_o = dramp.tile([C, 128], fp8, addr_space="Shared", name="dmy_o")
            nc.gpsimd.collective_compute(
                "AllGather",
                ALU.bypass,
                replica_groups=[list(range(C))],
                ins=[dmy_i.opt()],
                outs=[dmy_o.opt()],
            )

        ident_f = const.tile([P, P], f32, name="identf")
        make_identity(nc, ident_f)
        ident_t = const.tile([P, P], tdt, name="identt")
        nc.vector.tensor_copy(ident_t, ident_f)

        imgT = pers.tile([P, T, KC, P], fp8, name="imgT")
        # gathered spec^T in arrival-major layout [q, src core, k, off]:
        # both unpack DMA sides are then 2KB-contiguous per partition
        specT = pers.tile([P, 2, C, KC, 512], fp8, name="specT")
        stage = [pers.tile([P, NL], fp8, name=f"stage{k}") for k in range(KC)]
        racc = pers.tile([P, N], bf16, name="racc")
        # one extra slot: the split second half of the very last tile
        rowacc = pers.tile([P, T, G + 1], f32, name="rowacc")
        ssi = pers.tile([P, T], f32, name="ssi")
        sss = pers.tile([P, T], f32, name="sss")
        rsi = pers.tile([P, T], f32, name="rsi")
        rss = pers.tile([P, T], f32, name="rss")
        rni16 = pers.tile([P, T], f32, name="rni16")
        rns16 = pers.tile([P, T], f32, name="rns16")
        dotd = pers.tile([P, T], f32, name="dotd")
        rows = pers.tile([P, T], f32, name="rows")

        # two chunked AllGathers: the mesh has a ~14us floor and meshes
        # serialize, but chunk 0 gates only the first two column groups.
        # Payload rows = SBUF partitions so the unpack runs are contiguous.
        cc_in = [dramp.tile([P, KC * 512], fp8, name=f"cc_in{q}") for q in range(2)]
        cc_out = [
            dramp.tile([C * P, KC * 512], fp8, addr_space="Shared", name=f"cc_out{q}")
            for q in range(2)
        ]

        # preload the sqrt table set before the first Square needs it
        warm = const.tile([P, 1], f32, name="actwarm")
        nc.vector.memset(warm, 1.0)
        nc.scalar.activation(warm, warm, FA.Sqrt)

        # ---- per chunk (2 tiles): load -> norms -> normalize(fp8) ->
        #      transpose -> DMA PSUM->cc_in -> AllGather.
        #      img loads are deferred so the spec DMAs go first.
        img_nat, spec_nat = [], [None] * T
        for th in range(2):
            hs = slice(4 * th, 4 * th + 4)
            for tt in range(4):
                t = 4 * th + tt
                st = natp.tile([P, D], bf16, tag="specnat")
                nc.sync.dma_start(st, spec.ap()[t * P : (t + 1) * P, :])
                spec_nat[t] = st
                s2 = scp.tile([P, D], f32, tag="scr")
                nc.scalar.activation(
                    s2, st, FA.Square, accum_out=sss[:, t : t + 1]
                )
            nc.vector.tensor_scalar_max(sss[:, hs], sss[:, hs], 1.0e-6)
            nc.vector.reciprocal(rss[:, hs], sss[:, hs])
            # 16/|row|: sqrt(256 * 1/ss)
            nc.scalar.activation(
                rns16[:, hs], rss[:, hs], FA.Sqrt, scale=FP8_PRESCALE**2
            )
            pt = ps.tile([P, 2048], tdt, tag="mm")
            for tt in range(4):
                t = 4 * th + tt
                sn = spn.tile([P, D], tdt, tag="specn")
                nc.vector.tensor_scalar_mul(sn, spec_nat[t], rns16[:, t : t + 1])
                for k in range(KC):
                    nc.tensor.transpose(
                        pt[:, 512 * k + 128 * tt : 512 * k + 128 * (tt + 1)],
                        sn[:, 128 * k : 128 * (k + 1)],
                        ident_t,
                    )
            for k in range(KC):
                nc.vector.tensor_copy(
                    stage[k][:, 512 * th : 512 * (th + 1)],
                    pt[:, 512 * k : 512 * (k + 1)],
                )
                nc.sync.dma_start(
                    cc_in[th][:, 512 * k : 512 * (k + 1)],
                    stage[k][:, 512 * th : 512 * (th + 1)],
                )
            if use_cc:
                nc.gpsimd.collective_compute(
                    "AllGather",
                    ALU.bypass,
                    replica_groups=[list(range(C))],
                    ins=[cc_in[th].opt()],
                    outs=[cc_out[th].opt()],
                )

        for t in range(T):
            it = natp.tile([P, D], bf16, tag="imgnat")
            nc.sync.dma_start(it, img.ap()[t * P : (t + 1) * P, :])
            img_nat.append(it)

        # ---- img norms + transpose (overlap the collective) ----
        for t in range(T):
            s1 = scp.tile([P, D], f32, tag="scr")
            nc.scalar.activation(
                s1, img_nat[t], FA.Square, accum_out=ssi[:, t : t + 1]
            )
        nc.vector.tensor_scalar_max(ssi, ssi, 1.0e-6)
        nc.vector.reciprocal(rsi, ssi)
        nc.scalar.activation(rni16, rsi, FA.Sqrt, scale=FP8_PRESCALE**2)
        for t in range(T):
            ig = spn.tile([P, D], tdt, tag="specn")
            nc.vector.tensor_scalar_mul(ig, img_nat[t], rni16[:, t : t + 1])
            pti = ps.tile([P, 512], tdt, tag="mm")
            for k in range(KC):
                nc.tensor.transpose(
                    pti[:, 128 * k : 128 * (k + 1)],
                    ig[:, 128 * k : 128 * (k + 1)],
                    ident_t,
                )
            # ACT does this copy: it idles during the collective window
            nc.scalar.copy(imgT[:, t, :, :], pti)

        # switch the ACT table set to exp while the collective runs.
        # Reading rni16 (written by the last Sqrt) pins this after the
        # sqrt-set activations so the scheduler cannot hoist it earlier.
        warm2 = const.tile([P, 1], f32, name="actwarm2")
        nc.scalar.activation(warm2, rni16[:, 0:1], FA.Exp, scale=-1.0)

        # diag dots on DVE fill the collective's shadow (raw operands)
        for t in range(T):
            s3 = scp.tile([P, D], f32, tag="scr")
            nc.vector.tensor_mul(out=s3, in0=img_nat[t], in1=spec_nat[t])
            nc.vector.reduce_sum(
                dotd[:, t : t + 1], s3, axis=mybir.AxisListType.X
            )
        # these outputs are final already — ship them in the dead window
        # instead of serializing behind the main loop's tail
        nc.sync.dma_start(dotd_o.ap(), dotd)
        nc.sync.dma_start(rni_o.ap(), rni16)
        nc.sync.dma_start(rns_o.ap(), rns16)

        # ---- load gathered spec^T. specT[:, q, r, k, off] <-> global spec
        # row 1024*r + 512*q + off (host only ever sums over all columns, so
        # the column permutation needs no host-side handling). One DMA per
        # (q, r): 2KB-contiguous runs on both sides.
        for q in range(2):
            for r in range(C):
                if use_cc:
                    src = cc_out[q][P * r : P * (r + 1), :]
                else:  # debug: replicate the local slice (numerically wrong)
                    src = cc_in[q][:, :]
                nc.sync.dma_start(specT[:, q, r, :, :], src)

        # ---- main loop: logits block, exp, row/col accumulation ----
        nc.vector.memset(rowacc[:, :, G : G + 1], 0.0)
        escale = scale / (FP8_PRESCALE * FP8_PRESCALE)
        with nc.allow_low_precision("bf16 exp-sum accumulation, error ~0.5% -> <1e-3 on loss"):
            for g in range(G):
                gsl = racc[:, GW * g : GW * (g + 1)]
                for m in range(T):
                    pm = ps.tile([P, GW], f32, tag="mm")
                    # fp8 DoubleRow: each matmul contracts 2 k-chunks (K=256)
                    for q in range(KC // 2):
                        for ns in range(GW // 512):
                            b = 4 * g + ns  # 512-col block = (qq, r) of specT
                            nc.tensor.matmul(
                                pm[:, 512 * ns : 512 * (ns + 1)],
                                imgT[:, m, 2 * q : 2 * q + 2, :],
                                specT[:, b // C, b % C, 2 * q : 2 * q + 2, :],
                                start=(q == 0),
                                stop=(q == KC // 2 - 1),
                                perf_mode=mybir.MatmulPerfMode.DoubleRow,
                            )
                    if m == 0:
                        nc.scalar.activation(
                            gsl, pm, FA.Exp,
                            scale=escale,
                            accum_out=rowacc[:, m, g : g + 1],
                        )
                    elif g == G - 1 and m == T - 1:
                        # last tile: exp/add/store pipeline in halves so the
                        # kernel tail is ~2us shorter
                        for h in range(2):
                            hsl = slice(1024 * h, 1024 * (h + 1))
                            e = ep.tile([P, 1024], bf16, tag="e")
                            nc.scalar.activation(
                                e, pm[:, hsl], FA.Exp,
                                scale=escale,
                                accum_out=rowacc[:, m, g + h : g + h + 1],
                            )
                            nc.vector.tensor_add(
                                out=gsl[:, hsl], in0=gsl[:, hsl], in1=e
                            )
                            nc.sync.dma_start(
                                racc_o.ap()[:, GW * g + 1024 * h :
                                            GW * g + 1024 * (h + 1)],
                                gsl[:, hsl],
                            )
                    else:
                        e = ep.tile([P, GW], bf16, tag="e")
                        nc.scalar.activation(
                            e, pm, FA.Exp,
                            scale=escale,
                            accum_out=rowacc[:, m, g : g + 1],
                        )
                        nc.vector.tensor_add(out=gsl, in0=gsl, in1=e)
                if not (g == G - 1):
                    # racc[g] complete: ship it out now, overlapping next g
                    nc.sync.dma_start(
                        racc_o.ap()[:, GW * g : GW * (g + 1)], gsl
                    )

        # ---- tails ----
        nc.vector.reduce_sum(rows, rowacc[:, :, :], axis=mybir.AxisListType.X)
        nc.sync.dma_start(rowsum_o.ap(), rows)

    nc.compile()
    return nc


def _ensure_ntff_hook():
    """antenv.axon_hooks is absent on this image; provide the tiny get/set
    registry and register trn_agent_boot's ctypes NTFF hook so trace=True
    works. Only used from test runs (KERNEL_TRACE=1)."""
    import sys
    import types

    try:
        import antenv.axon_hooks  # noqa: F401
        return
    except ImportError:
        pass
    mod = types.ModuleType("antenv.axon_hooks")
    _state = {"hook": None}
    mod.set_axon_ntff_profile_hook = lambda h: _state.__setitem__("hook", h)
    mod.get_axon_ntff_profile_hook = lambda: _state["hook"]
    import antenv

    sys.modules["antenv.axon_hooks"] = mod
    antenv.axon_hooks = mod
    try:
        from trn_agent_boot.trn_boot import _ntff_profile_via_ctypes

        mod.set_axon_ntff_profile_hook(
            _ntff_profile_via_ctypes("/opt/axon/libaxon_pjrt.so")
        )
    except Exception as e:  # degrade to no tracing
        print(f"NTFF hook setup failed: {e}")


def kernel(image_features, spectrum_features, logit_scale):
    scale = float(np.asarray(logit_scale))
    key = round(scale, 9)
    if key not in _cache:
        _cache[key] = _build(scale)
    nc = _cache[key]

    import ml_dtypes

    img = np.ascontiguousarray(
        np.asarray(image_features, dtype=np.float32).astype(ml_dtypes.bfloat16)
    )
    spec = np.ascontiguousarray(
        np.asarray(spectrum_features, dtype=np.float32).astype(ml_dtypes.bfloat16)
    )
    in_maps = [
        {"img": img[c * NL : (c + 1) * NL], "spec": spec[c * NL : (c + 1) * NL]}
        for c in range(C)
    ]
    trace = os.environ.get("KERNEL_TRACE") == "1"
    if trace:
        _ensure_ntff_hook()
    res = run_bass_kernel_spmd(nc, in_maps, core_ids=list(range(C)), trace=trace)
    if trace:
        print(f"HW exec time: {res.exec_time_ns} ns (mean {res.mean_exec_time_ns})")

    rs = np.stack([r["rowsum"] for r in res.results]).astype(np.float64)   # [C,P,T]
    cs = np.stack(
        [r["racc_o"].astype(np.float64).sum(axis=0) for r in res.results]
    )  # [C,N]
    dd = np.stack([r["dotd"] for r in res.results]).astype(np.float64)
    ri = np.stack([r["rni"] for r in res.results]).astype(np.float64)
    rr = np.stack([r["rns"] for r in res.results]).astype(np.float64)

    # rni/rns outputs carry the x16 fp8 prescale each
    diag_sum = float(np.sum(scale * dd * ri * rr)) / (FP8_PRESCALE * FP8_PRESCALE)
    lse_i_sum = float(np.sum(np.log(rs)))
    col_total = cs.sum(axis=0)  # still in device (chunk-major) column order
    lse_s_sum = float(np.sum(np.log(col_total)))
    loss = 0.5 * ((lse_i_sum - diag_sum) / N + (lse_s_sum - diag_sum) / N)
    return np.float32(loss)



# revision 2
# speedup vs baseline: 1.0245x; 1.0245x over previous
"""CLIP loss kernel for trn2, 8 NeuronCores, collective-free "flipped-wide" v4.

Grid: 8 cores = 4 img column blocks (cb = c//2, 2048 cols) x 2 spec row halves
(rh = c%2, 4096 rows). Each core computes the transposed logits block
out_T[4096 spec rows, 2048 img cols] with NO inter-core communication:

  - spec arrives HOST-TRANSPOSED as raw fp8 (x8): layout prep only. Its
    normalization happens inside the exp via a per-partition scale AP,
    rns_p = s/(16*sqrt(gram_p)); gram (block-diag of specT.T @ specT) is
    computed on PE in 4 octets (one pre-loop, three streamed into the loop)
    and extracted per 128-block with one scalar_tensor_tensor(x identity,
    accum_out); rsqrt runs on DVE (quake bit trick + 2 Newton steps) so the
    ACT exp table is never evicted.
  - img is fp8 (x4); its per-row norms come from squares split DVE/ACT, and
    the normalize (16/|x|) is fused into the PE transpose as a diagonal
    matmul (mixed fp8 x bf16). One f32->fp8 cast per 4-tile group,
    alternating DVE/ACT.
  - main loop per spec chunk m: 8 fp8 DoubleRow matmuls -> PSUM [128,2048]
    f32 -> ACT Exp (scale AP; accum_out = spec-axis exp row sums) -> DVE
    adds accumulate img-axis partial column sums in bf16. The first two
    tiles stream in halves so ACT starts ~5us earlier.
  - positive-pair logits are NOT recomputed: each tile's candidate diagonal
    block (col 128*(m%16), uniform across cores) is extracted from the pm
    PSUM (m<2) or the exp tile (m>=2) with one stt; the host keeps the
    entries that are true diagonals for that core.

Host: sums per-core partials in f64, takes logs -> scalar loss.
"""

import os
from contextlib import ExitStack

import numpy as np

import concourse.bass as bass
import concourse.mybir as mybir
from concourse import bacc, tile
from concourse.bass_utils import run_bass_kernel_spmd
from concourse.masks import make_identity

N, D, C = 8192, 512, 8
P = 128
KC = D // P          # 4 k-chunks
NI = 2048            # img cols per core
NS = 4096            # spec rows per core
TI = NI // P         # 16 img tiles
TS = NS // P         # 32 spec chunks (main-loop tiles)

f32 = mybir.dt.float32
bf16 = mybir.dt.bfloat16
fp8 = mybir.dt.float8e4
i32 = mybir.dt.int32
FA = mybir.ActivationFunctionType
ALU = mybir.AluOpType

PS_IMG = 16.0    # imgT fp8 prescale (x-hat * 16 after normalize)
PS_IN = 4.0      # raw img fp8 prescale (x * 4)
PS_SPT = 8.0     # spec fp8 prescale (y * 8)
MAGIC = 0x5F3759DF

_cache: dict = {}


def _build(scale: float):
    nc = bacc.Bacc("TRN2", target_bir_lowering=False, debug=False, num_devices=C)
    img = nc.dram_tensor("img", [NI, D], fp8, kind="ExternalInput")
    spt = nc.dram_tensor("spt", [KC, P, NS], fp8, kind="ExternalInput")
    rowacc_o = nc.dram_tensor("rowacc", [P, TS + 3], f32, kind="ExternalOutput")
    racc_o = nc.dram_tensor("racc_o", [P, NI], bf16, kind="ExternalOutput")
    gram_o = nc.dram_tensor("gram_o", [P, TS], f32, kind="ExternalOutput")
    dote_o = nc.dram_tensor("dote", [P, TS], f32, kind="ExternalOutput")

    with tile.TileContext(nc) as tc, ExitStack() as ctx:
        const = ctx.enter_context(tc.tile_pool(name="const", bufs=1))
        pers = ctx.enter_context(tc.tile_pool(name="pers", bufs=1))
        natp = ctx.enter_context(tc.tile_pool(name="nat", bufs=4))
        scp = ctx.enter_context(tc.tile_pool(name="scr", bufs=2))
        dgp = ctx.enter_context(tc.tile_pool(name="dg", bufs=16))
        ep = ctx.enter_context(tc.tile_pool(name="e", bufs=3))
        ps = ctx.enter_context(tc.tile_pool(name="ps", bufs=2, space="PSUM"))

        identf = const.tile([P, P], f32, name="identf")
        make_identity(nc, identf)
        identb = const.tile([P, P], bf16, name="identb")
        nc.vector.tensor_copy(identb, identf)
        onesb = const.tile([P, 1], bf16, name="onesb")
        nc.vector.memset(onesb, 1.0)

        # ACT: warm sqrt table (used by rni + ACT squares); exp table later
        warm = const.tile([P, 1], f32, name="warm")
        nc.vector.memset(warm, 1.0)
        nc.scalar.activation(warm, warm, FA.Sqrt)

        # ---- PE warmup ----
        pw = ps.tile([P, NI], bf16, tag="mm")
        for i in range(18):
            nc.tensor.transpose(pw[:, :P], identb, identb)

        # ---- input DMAs ----
        img_nat = []
        img_dmas = [
            img.ap()[512 * g : 512 * (g + 1), :].rearrange("(t p) d -> p t d", p=P)
            for g in range(4)
        ]
        for g in range(4):
            it = natp.tile([P, 4, D], fp8, tag="imgnat", name=f"imgnat{g}")
            img_nat.append(it)
        sptt = pers.tile([P, KC, NS], fp8, name="sptt")
        nc.sync.dma_start(img_nat[0], img_dmas[0])
        nc.sync.dma_start(sptt[:, :, :1024], spt.ap()[:, :, :1024])
        nc.sync.dma_start(img_nat[1], img_dmas[1])
        nc.sync.dma_start(img_nat[2], img_dmas[2])
        nc.sync.dma_start(img_nat[3], img_dmas[3])
        for j in range(1, 4):
            nc.sync.dma_start(
                sptt[:, :, 1024 * j : 1024 * (j + 1)],
                spt.ap()[:, :, 1024 * j : 1024 * (j + 1)],
            )

        # ---- img norms ----
        ssi = pers.tile([P, TI], f32, name="ssi")
        rsi = pers.tile([P, TI], f32, name="rsi")
        rni = pers.tile([P, TI], f32, name="rni")
        diags = [None] * TI

        def sq_dve(lo, hi):
            for t in range(lo, hi):
                sq = scp.tile([P, D], bf16, tag="sq")
                nc.vector.scalar_tensor_tensor(
                    out=sq, in0=img_nat[t // 4][:, t % 4, :], scalar=1.0,
                    in1=img_nat[t // 4][:, t % 4, :],
                    op0=ALU.mult, op1=ALU.mult,
                    accum_out=ssi[:, t : t + 1],
                )

        def sq_act(lo, hi):
            for t in range(lo, hi):
                sq = scp.tile([P, D], f32, tag="asq")
                nc.scalar.activation(
                    sq, img_nat[t // 4][:, t % 4, :], FA.Square,
                    accum_out=ssi[:, t : t + 1],
                )

        def rni_batch(b):
            hs = slice(4 * b, 4 * b + 4)
            nc.vector.tensor_scalar_max(rsi[:, hs], ssi[:, hs], 1e-6)
            nc.vector.reciprocal(rsi[:, hs], rsi[:, hs])
            nc.scalar.activation(
                rni[:, hs], rsi[:, hs], FA.Sqrt, scale=PS_IMG * PS_IMG
            )

        def diag_batch(b):
            for t in range(4 * b, 4 * b + 4):
                dg = dgp.tile([P, P], bf16, tag="diag")
                nc.vector.tensor_scalar_mul(dg, identb, rni[:, t : t + 1])
                diags[t] = dg

        sq_dve(0, 4)
        rni_batch(0)
        diag_batch(0)
        sq_dve(4, 8)
        rni_batch(1)
        diag_batch(1)
        sq_act(8, 12)
        rni_batch(2)
        diag_batch(2)
        sq_act(12, 16)
        rni_batch(3)
        diag_batch(3)

        # ---- gram machinery ----
        gram = pers.tile([P, TS], f32, name="gram")
        rns = pers.tile([P, TS], f32, name="rns")
        cmagic = const.tile([P, 8], i32, name="cmagic")
        nc.vector.memset(cmagic, MAGIC)
        gh = pers.tile([P, TS], f32, name="gh")
        gi = pers.tile([P, 8], i32, name="gi")
        gt1 = pers.tile([P, 8], f32, name="gt1")
        gy = pers.tile([P, 8], f32, name="gy")

        def gram_mms(o):
            pg = ps.tile([P, 8 * P], f32, tag="mm")
            for b in range(8):
                m = 8 * o + b
                for q in range(KC // 2):
                    nc.tensor.matmul(
                        pg[:, P * b : P * (b + 1)],
                        sptt[:, 2 * q : 2 * q + 2, P * m : P * (m + 1)],
                        sptt[:, 2 * q : 2 * q + 2, P * m : P * (m + 1)],
                        start=(q == 0),
                        stop=(q == KC // 2 - 1),
                        perf_mode=mybir.MatmulPerfMode.DoubleRow,
                    )
            return pg

        def gram_extract(o, pg):
            hs = slice(8 * o, 8 * o + 8)
            for b in range(8):
                m = 8 * o + b
                junk = scp.tile([P, P], f32, tag="ext")
                nc.vector.scalar_tensor_tensor(
                    out=junk, in0=pg[:, P * b : P * (b + 1)], scalar=1.0,
                    in1=identb, op0=ALU.mult, op1=ALU.mult,
                    accum_out=gram[:, m : m + 1],
                )
            nc.vector.tensor_scalar_max(gh[:, hs], gram[:, hs], 1e-6)
            nc.vector.tensor_scalar(
                out=gi, in0=gh[:, hs].bitcast(i32), scalar1=1, scalar2=None,
                op0=ALU.logical_shift_right,
            )
            nc.vector.tensor_tensor(out=gi, in0=cmagic, in1=gi, op=ALU.subtract)
            nc.vector.tensor_scalar_mul(gh[:, hs], gh[:, hs], 0.5)
            y0 = gi[:, :].bitcast(f32)
            for it in range(2):
                yin = y0 if it == 0 else gy
                nc.vector.tensor_mul(out=gt1, in0=yin, in1=yin)
                nc.vector.tensor_mul(out=gt1, in0=gt1, in1=gh[:, hs])
                nc.vector.tensor_scalar(
                    out=gt1, in0=gt1, scalar1=-1.0, op0=ALU.mult,
                    scalar2=1.5, op1=ALU.add,
                )
                nc.vector.tensor_tensor(out=gy, in0=yin, in1=gt1, op=ALU.mult)
            nc.vector.tensor_scalar_mul(rns[:, hs], gy, scale / PS_IMG)

        # gram octet 0 (needs spt cols 0-1023 only) + its rns, pre-everything
        pg0 = gram_mms(0)
        gram_extract(0, pg0)

        # ---- scale-fused transposes; k-major psum; contiguous casts ----
        imgTh = [
            pers.tile([P, 2, KC, 512], fp8, name=f"imgT{h}") for h in range(2)
        ]
        for g in range(4):
            ptg = ps.tile([P, 2048], f32, tag="mm")
            for tt in range(4):
                t = 4 * g + tt
                for k in range(KC):
                    nc.tensor.matmul(
                        ptg[:, 512 * k + P * tt : 512 * k + P * (tt + 1)],
                        img_nat[g][:, tt, P * k : P * (k + 1)],
                        diags[t],
                        start=True,
                        stop=True,
                    )
            h, q2 = g // 2, g % 2
            dst = imgTh[h][:, q2, :, :]
            if g % 2 == 0:
                nc.vector.tensor_copy(dst, ptg)
            else:
                nc.scalar.copy(dst, ptg)
            if g == 1:
                warm2 = const.tile([P, 1], f32, name="warm2")
                nc.scalar.activation(
                    warm2, rni[:, TI - 1 : TI], FA.Exp, scale=-1e-9
                )
        # keep the PE's HAM activity window busy across the cast gap
        for i in range(8):
            nc.tensor.transpose(pw[:, :P], identb, identb)

        # ---- main loop ----
        racc = pers.tile([P, NI], bf16, name="racc")
        rowacc = pers.tile([P, TS + 3], f32, name="rowacc")
        dote = pers.tile([P, TS], f32, name="dote")

        def diag_candidate(m, pm, e_lo=None, e_hi=None):
            """Extract the candidate diag block of tile m (col 128*(m%16)).
            For m<2 read the f32 PSUM; else read the bf16 exp tiles."""
            coff = P * (m % 16)
            junk = scp.tile([P, P], f32, tag="dx")
            if m < 2:
                src = pm[:, coff : coff + P]
            elif e_hi is not None and coff >= 1024:
                src = e_hi[:, coff - 1024 : coff - 1024 + P]
            elif e_lo is not None:
                src = e_lo[:, coff : coff + P]
            else:
                src = pm[:, coff : coff + P]
            nc.vector.scalar_tensor_tensor(
                out=junk, in0=src, scalar=1.0, in1=identb,
                op0=ALU.mult, op1=ALU.mult,
                accum_out=dote[:, m : m + 1],
            )

        with nc.allow_low_precision("bf16 exp-sum accumulation, ~0.5% -> <1e-3 on loss"):
            for m in range(TS):
                pm = ps.tile([P, NI], f32, tag="mm")

                def mms(nblo, nbhi, m=m, pm=pm):
                    for q in range(KC // 2):
                        for nb in range(nblo, nbhi):
                            nc.tensor.matmul(
                                pm[:, 512 * nb : 512 * (nb + 1)],
                                sptt[:, 2 * q : 2 * q + 2, P * m : P * (m + 1)],
                                imgTh[nb // 2][:, nb % 2, 2 * q : 2 * q + 2, :],
                                start=(q == 0),
                                stop=(q == KC // 2 - 1),
                                perf_mode=mybir.MatmulPerfMode.DoubleRow,
                            )

                if m in (0, 1):
                    # stream halves; slot the next gram octet into the gap
                    mms(0, 2)
                    if m == 0:
                        nc.scalar.activation(
                            racc[:, :1024], pm[:, :1024], FA.Exp,
                            scale=rns[:, m : m + 1],
                            accum_out=rowacc[:, m : m + 1],
                        )
                    else:
                        e = ep.tile([P, 1024], bf16, tag="e")
                        nc.scalar.activation(
                            e, pm[:, :1024], FA.Exp,
                            scale=rns[:, m : m + 1],
                            accum_out=rowacc[:, m : m + 1],
                        )
                        nc.vector.tensor_add(
                            out=racc[:, :1024], in0=racc[:, :1024], in1=e
                        )
                    diag_candidate(m, pm)
                    pgx = gram_mms(m + 1)
                    mms(2, 4)
                    gram_extract(m + 1, pgx)
                    if m == 0:
                        nc.scalar.activation(
                            racc[:, 1024:], pm[:, 1024:], FA.Exp,
                            scale=rns[:, m : m + 1],
                            accum_out=rowacc[:, TS + 1 + m : TS + 2 + m],
                        )
                    else:
                        e = ep.tile([P, 1024], bf16, tag="e")
                        nc.scalar.activation(
                            e, pm[:, 1024:], FA.Exp,
                            scale=rns[:, m : m + 1],
                            accum_out=rowacc[:, TS + 1 + m : TS + 2 + m],
                        )
                        nc.vector.tensor_add(
                            out=racc[:, 1024:], in0=racc[:, 1024:], in1=e
                        )
                    continue

                mms(0, 4)
                if m == TS - 1:
                    e_parts = []
                    for h in range(2):
                        hsl = slice(1024 * h, 1024 * (h + 1))
                        e = ep.tile([P, 1024], bf16, tag="e")
                        nc.scalar.activation(
                            e, pm[:, hsl], FA.Exp,
                            scale=rns[:, m : m + 1],
                            accum_out=rowacc[:, m + h : m + h + 1],
                        )
                        e_parts.append(e)
                        nc.vector.tensor_add(
                            out=racc[:, hsl], in0=racc[:, hsl], in1=e
                        )
                        nc.sync.dma_start(racc_o.ap()[:, hsl], racc[:, hsl])
                    diag_candidate(m, pm, e_lo=e_parts[0], e_hi=e_parts[1])
                else:
                    e = ep.tile([P, NI], bf16, tag="e")
                    nc.scalar.activation(
                        e, pm, FA.Exp,
                        scale=rns[:, m : m + 1],
                        accum_out=rowacc[:, m : m + 1],
                    )
                    nc.vector.tensor_add(out=racc, in0=racc, in1=e)
                    diag_candidate(m, pm, e_lo=e)
                if m == 2:
                    pgx = gram_mms(3)
                    gram_extract(3, pgx)

        nc.sync.dma_start(rowacc_o.ap(), rowacc)
        nc.sync.dma_start(gram_o.ap(), gram)
        nc.sync.dma_start(dote_o.ap(), dote)

    nc.compile()
    return nc


def _ensure_ntff_hook():
    """antenv.axon_hooks shim so trace=True works on this image."""
    import sys
    import types

    try:
        import antenv.axon_hooks  # noqa: F401
        return
    except ImportError:
        pass
    mod = types.ModuleType("antenv.axon_hooks")
    _state = {"hook": None}
    mod.set_axon_ntff_profile_hook = lambda h: _state.__setitem__("hook", h)
    mod.get_axon_ntff_profile_hook = lambda: _state["hook"]
    import antenv

    sys.modules["antenv.axon_hooks"] = mod
    antenv.axon_hooks = mod
    try:
        from trn_agent_boot.trn_boot import _ntff_profile_via_ctypes

        mod.set_axon_ntff_profile_hook(
            _ntff_profile_via_ctypes("/opt/axon/libaxon_pjrt.so")
        )
    except Exception as e:  # degrade to no tracing
        print(f"NTFF hook setup failed: {e}")


def kernel(image_features, spectrum_features, logit_scale):
    import ml_dtypes

    scale = float(np.asarray(logit_scale))
    key = round(scale, 9)
    if key not in _cache:
        _cache[key] = _build(scale)
    nc = _cache[key]

    imgf = np.asarray(image_features, dtype=np.float32)
    spef = np.asarray(spectrum_features, dtype=np.float32)
    img8 = (imgf * PS_IN).astype(ml_dtypes.float8_e4m3)
    sptT = np.ascontiguousarray(
        (spef.T * PS_SPT).astype(ml_dtypes.float8_e4m3).reshape(KC, P, N)
    )

    in_maps = []
    for c in range(C):
        cb, rh = c // 2, c % 2
        in_maps.append({
            "img": np.ascontiguousarray(img8[NI * cb : NI * (cb + 1)]),
            "spt": np.ascontiguousarray(sptT[:, :, NS * rh : NS * (rh + 1)]),
        })

    trace = os.environ.get("KERNEL_TRACE") == "1"
    if trace:
        _ensure_ntff_hook()
    res = run_bass_kernel_spmd(nc, in_maps, core_ids=list(range(C)), trace=trace)
    if trace:
        print(f"HW exec time: {res.exec_time_ns} ns (mean {res.mean_exec_time_ns})")

    # ---- host reduction (f64) ----
    rowS = np.zeros(N)
    colS = np.zeros(N)
    for c in range(C):
        cb, rh = c // 2, c % 2
        r = res.results[c]
        ra = r["rowacc"].astype(np.float64)       # [P, TS+3]
        ra[:, TS - 1] += ra[:, TS]
        ra[:, 0] += ra[:, TS + 1]
        ra[:, 1] += ra[:, TS + 2]
        rowS[NS * rh : NS * (rh + 1)] += ra[:, :TS].T.ravel()
        racc = r["racc_o"].astype(np.float64)
        colS[NI * cb : NI * (cb + 1)] += racc.sum(axis=0)

    # gram -> |y| (for rns_host) from one core per rh
    grams = {}
    for c, rh in ((0, 0), (1, 1)):
        grams[rh] = res.results[c]["gram_o"].astype(np.float64)   # [P, TS]

    # diag: core (cb, rh) holds true diagonals for m in [16*(cb%2), +16)
    # iff rh == cb // 2; candidate col = 128*(m%16), value dote[p, m].
    diag_sum = 0.0
    for c in range(C):
        cb, rh = c // 2, c % 2
        if rh != cb // 2:
            continue
        de = res.results[c]["dote"].astype(np.float64)   # [P, TS]
        g = grams[rh]
        m0 = 16 * (cb % 2)
        for m in range(m0, m0 + 16):
            if m < 2:
                # raw pm dot: l_ii = pmdiag * s / (16*sqrt(gram))
                rns_h = scale / (16.0 * np.sqrt(np.maximum(g[:, m], 1e-12)))
                diag_sum += float(np.sum(de[:, m] * rns_h))
            else:
                # exp values: l_ii = log(e)
                diag_sum += float(np.sum(np.log(np.maximum(de[:, m], 1e-30))))

    lse_s_sum = float(np.sum(np.log(rowS)))
    lse_i_sum = float(np.sum(np.log(colS)))
    loss = 0.5 * (lse_i_sum + lse_s_sum) / N - diag_sum / N
    return np.float32(loss)
